# revision 27
# baseline (speedup 1.0000x reference)
"""Trainium2 Bass kernel for nn_MultiHeadAttention_7516192768586.

Full MHA: QKV projection -> masked softmax attention -> merge heads ->
residual add -> LayerNorm.  B=2, T=2048, D=1024, 16 heads (depth 64).

Sharding (8 cores): 2 batches x 4 head-groups (4 heads each, i.e. a
256-channel slice of the projected dims).  Each core computes attention for
its 4 heads, the residual+LayerNorm for its 256 output *columns* over all
2048 rows of its batch; LayerNorm statistics (sum x, sum x^2 over the full
1024 channels) are combined with a tiny 8KB-per-chunk AllReduce within each
batch-group of 4 cores.  Host reassembles the 8 (2048, 256) column slices.

Key points:
  * q/k/v fed transposed (contraction dim D on partitions) as bf16; v is
    fed s-tile-major so compute can start as soon as the first block lands
  * masked keys are compacted away on the host; compaction padding columns
    are all-zero so they contribute exp(0)*0 = 0 to both the context
    numerator and the denominator (no pad bias needed at all)
  * the reference's jnp.repeat(mask, h, axis=0) is batch-major while its
    head stacking is head-major, so attention row (head eta, batch beta)
    is masked by mask[eta // 8] -- replicated here faithfully
  * V's weight matrix gets one extra "ones" output channel per head so the
    attention-context matmul also produces the softmax denominators
  * V's bias is folded into the residual (ctx = sum attn (V+bv) = ctx0 +
    bv since attn rows sum to 1)
  * exp is split between the Scalar engine (exact, table-based) and the
    Vector engine (Schraudolph bit-trick: int16(x*s+b) bitcast to bf16,
    ~1.6% mean error -- harmless after the softmax normalization since the
    output tolerance is 2e-2 and ctx ~ 0.05 x output scale)
  * score matmuls for the two heads of a K-partition (rows 0-63 / 64-127)
    are issued back-to-back so the PE runs them concurrently in distinct
    row-groups
  * LayerNorm is pipelined per 1024-row chunk: fused scalar_tensor_tensor
    ops (with accum_out producing sum-x / sum-x^2 for free) run on the
    otherwise idle GpSimd engine, the per-chunk AllReduce overlaps the
    next chunk's attention, and 1/sqrt(var+eps) is exp(-0.5*ln(var+eps))
    so only one activation-table set is ever loaded
"""

import sys

if "/opt/trn_rl_repo" not in sys.path:
    sys.path.insert(0, "/opt/trn_rl_repo")

import contextlib

import ml_dtypes
import numpy as np

import bass_rust as _br
import concourse.bass as bass
import concourse.tile as tile
from concourse import mybir
from concourse.bass_utils import run_bass_kernel_spmd
from concourse.vector_clock import ScopedClock

F32 = mybir.dt.float32
BF16 = mybir.dt.bfloat16
I16 = mybir.dt.int16
BF = ml_dtypes.bfloat16

NUM_HEADS = 16
LN_EPS = 1e-5
B, T, D = 2, 2048, 1024
DEPTH = D // NUM_HEADS  # 64
HPC = 4  # heads per core
DD = HPC * DEPTH  # 256 projected channels per core
DV = 1152  # v contraction dim padded: 1024 + ones row + zeros (9 k-tiles)
NKV = DV // 128  # 9
TT = T // 128  # 16 t-tiles
AluOp = mybir.AluOpType
Act = mybir.ActivationFunctionType

# Schraudolph exp on bf16 bit patterns: e^(x/8) ~= bf16_bits(int16(x*ES + EB))
ES = 0.125 * 128.0 * float(np.log2(np.e))
EB = 127.0 * 128.0 - 7.0


class _TC(tile.TileContext):
    """TileContext whose tail drain splits its sem waits across 1-wait NOPs
    (this walrus build rejects >1 sync wait on one instruction)."""

    def _drain_and_barrier(self, tick_clock, wait_clock):
        nc = self.nc
        drain_inst = nc.sync.drain()
        wait_clock.add_sem_waits(
            drain_inst.ins, ScopedClock({None: tick_clock.global_clock})
        )
        si = drain_inst.ins.sync_info
        waits = list(si.on_wait) if si is not None and si.on_wait else []
        if len(waits) > 1:
            si.on_wait = waits[:1]
            for i in range(1, len(waits)):
                extra = nc.sync.nop()
                extra.ins.sync_info = _br.SyncInfo(
                    on_wait=waits[i : i + 1], on_update=[]
                )
        nc.all_engine_barrier()
        popped = nc._tile_sem_poison_stack.pop()
        assert popped is self._sem_poison
        assert self.sems is not None
        nc.clear_and_free_semaphores(list(self.sems.allocated().values()))
        nc.all_engine_barrier()


def _split_multi_waits(nc):
    """Move extra sem waits (>1 per instruction) onto same-engine NOPs
    inserted immediately before the instruction."""
    f = nc.m.functions[0]
    cur_bb = nc.cur_bb
    for block in f.blocks:
        insts = list(block.instructions)
        if not any(
            i.sync_info is not None
            and i.sync_info.on_wait
            and len(i.sync_info.on_wait) > 1
            for i in insts
        ):
            continue
        new_list = []
        for inst in insts:
            si = inst.sync_info
            if si is not None and si.on_wait and len(si.on_wait) > 1:
                waits = list(si.on_wait)
                si.on_wait = waits[:1]
                for w in waits[1:]:
                    eng = nc.engines[inst.engine]
                    nop = eng.nop()
                    tail_bb = cur_bb.bb if hasattr(cur_bb, "bb") else cur_bb
                    tl = list(tail_bb.instructions)
                    assert tl and tl[-1].name == nop.ins.name
                    tail_bb.instructions = tl[:-1]
                    nop.ins.sync_info = _br.SyncInfo(on_wait=[w], on_update=[])
                    new_list.append(nop.ins)
            new_list.append(inst)
        block.instructions = new_list


def _build(SP):
    """Build the per-core Bass program. SP = padded compacted key count."""
    NS = SP // 128  # s-tiles
    NKCH = (SP + 511) // 512  # 512-wide chunks of SP for the K projection

    nc = bass.Bass("TRN2", target_bir_lowering=False, debug=False, num_devices=8)

    p = lambda name, shape, dt: nc.declare_dram_parameter(name, shape, dt, isOutput=False)
    qT = p("qT", [D, T], BF16)
    kT = p("kT", [D, SP], BF16)
    vTs = p("vTs", [NS * DV, 128], BF16)  # s-tile-major blocks of v^T
    wqT = p("wqT", [D, DD], BF16)
    wkT = p("wkT", [D, DD], BF16)
    wvT = p("wvT", [DV, HPC * (DEPTH + 1)], BF16)
    bq = p("bq", [128, 2], F32)
    bk = p("bk", [128, 2], F32)
    qres = p("qres", [T, DD], BF16)
    gam = p("gamma", [1, DD], BF16)
    bet = p("beta", [1, DD], BF16)
    out = nc.declare_dram_parameter("out", [T, DD], BF16, isOutput=True)

    with _TC(nc) as tc, contextlib.ExitStack() as ctx:
        singles = ctx.enter_context(tc.tile_pool(name="singles", bufs=1))
        persist = ctx.enter_context(tc.tile_pool(name="persist", bufs=1))
        work = ctx.enter_context(tc.tile_pool(name="work", bufs=4))
        dram = ctx.enter_context(tc.tile_pool(name="dram", bufs=1, space="DRAM"))

        # ---- constants / weights (in consumption order) ----
        wv_sb = singles.tile([128, NKV, HPC * (DEPTH + 1)], BF16)
        nc.sync.dma_start(out=wv_sb[:], in_=wvT[:].rearrange("(kt p) c -> p kt c", p=128))
        bq_sb = singles.tile([128, 2], F32)
        nc.gpsimd.dma_start(out=bq_sb[:], in_=bq[:])
        bk_sb = singles.tile([128, 2], F32)
        nc.gpsimd.dma_start(out=bk_sb[:], in_=bk[:])
        gam_sb = singles.tile([128, DD], BF16)
        g_ap = gam[:]
        nc.gpsimd.dma_start(
            out=gam_sb[:],
            in_=bass.AP(tensor=g_ap.tensor, offset=g_ap.offset, ap=[[0, 128], list(g_ap.ap[-1])]),
        )
        bet_sb = singles.tile([128, DD], BF16)
        b_ap = bet[:]
        nc.gpsimd.dma_start(
            out=bet_sb[:],
            in_=bass.AP(tensor=b_ap.tensor, offset=b_ap.offset, ap=[[0, 128], list(b_ap.ap[-1])]),
        )
        eps_sb = singles.tile([128, 1], F32)
        nc.vector.memset(eps_sb[:], LN_EPS)
        # warm the scalar-engine exp table during the initial DMA wait
        warm = singles.tile([128, 1], F32)
        nc.vector.memset(warm[:], 1.0)
        nc.scalar.activation(out=warm[:], in_=warm[:], func=Act.Exp)
        # warm the collective path: the first AllReduce of a NEFF pays a
        # ~11.5us setup delay; also acts as a soft cross-core sync
        warm_d = dram.tile([128, 1], F32, name="warmd")
        warm_a = dram.tile([128, 1], F32, name="warma")
        nc.sync.dma_start(out=warm_d[:], in_=warm[:])
        nc.gpsimd.collective_compute(
            "AllReduce",
            AluOp.add,
            replica_groups=[[0, 1, 2, 3], [4, 5, 6, 7]],
            ins=[warm_d[:].opt()],
            outs=[warm_a[:].opt()],
        )

        # ---- persistent activations ----
        QT_sb = persist.tile([128, 2, T], BF16)  # [dd-in-tile, ddt, t]
        KT_sb = persist.tile([128, 2, SP], BF16)
        VH_sb = persist.tile([128, NS, HPC * (DEPTH + 1)], BF16)  # [s, st, head*65+c]
        ctxT_sb = persist.tile([128, HPC, T], BF16)  # rows 0..64 valid
        nc.gpsimd.memset(ctxT_sb[:], 0.0)
        ctxn_sb = persist.tile([128, HPC, TT, 128], BF16)  # transposed ctx
        x_sb = persist.tile([128, TT, DD], BF16)  # residual+ctx, pre-norm
        out_sb = persist.tile([128, TT, DD], BF16)  # normalized output
        mu = singles.tile([128, TT], F32)
        rstd = singles.tile([128, TT], F32)

        # ---- input streaming: all big input DMAs up front (sync/HWDGE),
        # in consumption order: vTs, wk, kT, wq, qT, qres ----
        ain = ctx.enter_context(tc.tile_pool(name="ain", bufs=8))
        vin = ctx.enter_context(tc.tile_pool(name="vin", bufs=NS))
        vts = []
        for st in range(NS):
            t_ = vin.tile([128, NKV, 128], BF16, tag="vin", name="vt")
            nc.sync.dma_start(
                out=t_[:],
                in_=vTs[DV * st : DV * (st + 1), :].rearrange(
                    "(kt p) s -> p kt s", p=128
                ),
            )
            vts.append(t_)
        wk_sb = singles.tile([128, 8, DD], BF16)
        nc.sync.dma_start(out=wk_sb[:], in_=wkT[:].rearrange("(kt p) c -> p kt c", p=128))
        kin = []
        for kt in range(8):
            t_ = ain.tile([128, SP], BF16, tag="kin", name="kin")
            nc.sync.dma_start(out=t_[:], in_=kT[128 * kt : 128 * (kt + 1), :])
            kin.append(t_)
        wq_sb = singles.tile([128, 8, DD], BF16)
        nc.sync.dma_start(out=wq_sb[:], in_=wqT[:].rearrange("(kt p) c -> p kt c", p=128))
        qin = []
        for kt in range(8):
            t_ = ain.tile([128, T], BF16, tag="qin", name="qin")
            nc.sync.dma_start(out=t_[:], in_=qT[128 * kt : 128 * (kt + 1), :])
            qin.append(t_)
        qres_sb = persist.tile([128, TT, DD], BF16)
        nc.sync.dma_start(
            out=qres_sb[:], in_=qres[:].rearrange("(tt p) c -> p tt c", p=128)
        )

        # ---- V projection (own PSUM scope, closed before attention) ----
        with tc.tile_pool(name="pv", bufs=2, space="PSUM") as pv:
            for st in range(NS):
                ps = pv.tile([128, HPC * (DEPTH + 1)], F32, tag="pv")
                for kt in range(NKV):
                    nc.tensor.matmul(
                        ps[:],
                        vts[st][:, kt, :],
                        wv_sb[:, kt, :],
                        start=(kt == 0),
                        stop=(kt == NKV - 1),
                    )
                if st % 2 == 0:
                    nc.vector.tensor_copy(VH_sb[:, st, :], ps[:])
                else:
                    nc.scalar.copy(VH_sb[:, st, :], ps[:])

        # ---- K/Q projections (own PSUM scope) ----
        with tc.tile_pool(name="pqk", bufs=3, space="PSUM") as pqk:

            def k_proj(ddt):
                for sch in range(NKCH):
                    w = min(512, SP - 512 * sch)
                    ps = pqk.tile([128, 512], F32, tag="pqk", name="kps")
                    for kt in range(8):
                        nc.tensor.matmul(
                            ps[:, :w],
                            wk_sb[:, kt, 128 * ddt : 128 * (ddt + 1)],
                            kin[kt][:, 512 * sch : 512 * sch + w],
                            start=(kt == 0),
                            stop=(kt == 7),
                        )
                    dst = KT_sb[:, ddt, 512 * sch : 512 * sch + w]
                    if sch % 2 == 0:
                        nc.vector.tensor_scalar(
                            out=dst, in0=ps[:, :w],
                            scalar1=bk_sb[:, ddt : ddt + 1], scalar2=None,
                            op0=AluOp.add,
                        )
                    else:
                        nc.scalar.activation(
                            out=dst, in_=ps[:, :w], func=Act.Identity,
                            bias=bk_sb[:, ddt : ddt + 1], scale=1.0,
                        )

            def q_proj(ddt):
                for tch in range(4):
                    ps = pqk.tile([128, 512], F32, tag="pqk", name="qps")
                    for kt in range(8):
                        nc.tensor.matmul(
                            ps[:],
                            wq_sb[:, kt, 128 * ddt : 128 * (ddt + 1)],
                            qin[kt][:, 512 * tch : 512 * (tch + 1)],
                            start=(kt == 0),
                            stop=(kt == 7),
                        )
                    dst = QT_sb[:, ddt, 512 * tch : 512 * (tch + 1)]
                    if tch % 2 == 0:
                        nc.scalar.activation(
                            out=dst, in_=ps[:], func=Act.Identity,
                            bias=bq_sb[:, ddt : ddt + 1], scale=1.0,
                        )
                    else:
                        nc.vector.tensor_scalar(
                            out=dst, in0=ps[:],
                            scalar1=bq_sb[:, ddt : ddt + 1], scalar2=None,
                            op0=AluOp.add,
                        )

            k_proj(0)
            q_proj(0)
            k_proj(1)
            q_proj(1)

        # ---- attention + pipelined LN ----
        stats_dram = [dram.tile([128, TT], F32, name=f"std{i}") for i in range(2)]
        ar_dram = [dram.tile([128, TT], F32, name=f"ard{i}") for i in range(2)]
        epool = ctx.enter_context(tc.tile_pool(name="epool", bufs=6))
        # PSUM: 3 score tiles (2 banks each, depth-3 pipeline) + 1 ctx (2) = 8
        pscore = ctx.enter_context(tc.tile_pool(name="pscore", bufs=3, space="PSUM"))
        pctx = ctx.enter_context(tc.tile_pool(name="pctx", bufs=1, space="PSUM"))

        # exp engine schedule: every act_mod-th tile on the scalar engine
        # (exact), the rest on the vector engine (bit-trick)
        ecnt = [0]

        def emit_exp(e_ap, s_ap, act_mod):
            i = ecnt[0]
            ecnt[0] += 1
            if i % act_mod == act_mod - 1:
                nc.scalar.activation(out=e_ap, in_=s_ap, func=Act.Exp, scale=0.125)
            else:
                nc.vector.tensor_scalar(
                    out=e_ap.bitcast(I16), in0=s_ap,
                    scalar1=ES, scalar2=EB,
                    op0=AluOp.mult, op1=AluOp.add,
                )

        def attn_head(hd, tc_, copy_eng, act_mod=3, fillers=()):
            """Attention for head hd over t-chunk tc_ (1024 queries).

            Software-pipelined: scores(st) and exp(st) are emitted before
            ctx(st-1) so the PE streams scores while ACT/DVE exponentiate
            and the 3-deep score-psum pool absorbs exp latency jitter.
            ``fillers``: closures emitted one-per-st (spread LN work from
            earlier chunks through this head's vector-engine stream)."""
            t0 = 1024 * tc_
            ddt, h = hd // 2, hd % 2
            r0 = DEPTH * h
            cps = pctx.tile([DEPTH + 1, 1024], F32, tag="cps", name="cps")
            es = [None] * NS
            fillers = list(fillers)

            def emit_ctx(st):
                for q2 in range(2):
                    nc.tensor.matmul(
                        cps[:, 512 * q2 : 512 * (q2 + 1)],
                        VH_sb[:, st, (DEPTH + 1) * hd : (DEPTH + 1) * (hd + 1)],
                        es[st][:, 512 * q2 : 512 * (q2 + 1)],
                        start=(st == 0),
                        stop=(st == NS - 1),
                    )

            for st in range(NS):
                s_ = pscore.tile([128, 1024], F32, tag="sc", name="s")
                for q2 in range(2):
                    nc.tensor.matmul(
                        s_[:, 512 * q2 : 512 * (q2 + 1)],
                        KT_sb[r0 : r0 + DEPTH, ddt, 128 * st : 128 * (st + 1)],
                        QT_sb[r0 : r0 + DEPTH, ddt, t0 + 512 * q2 : t0 + 512 * (q2 + 1)],
                        start=True,
                        stop=True,
                    )
                e_ = epool.tile([128, 1024], BF16, tag="e")
                emit_exp(e_[:], s_[:], act_mod)
                es[st] = e_
                if st > 0:
                    emit_ctx(st - 1)
                if fillers:
                    fillers.pop(0)()
            emit_ctx(NS - 1)
            for f in fillers:
                f()

            dst = ctxT_sb[0 : DEPTH + 1, hd, t0 : t0 + 1024]
            if copy_eng == "v":
                nc.vector.tensor_copy(dst, cps[:])
            else:
                nc.scalar.copy(dst, cps[:])
            nc.sync.dma_start_transpose(
                ctxn_sb[:, hd, 8 * tc_ : 8 * (tc_ + 1), :],
                ctxT_sb[:, hd, t0 : t0 + 1024],
            )

        def phase5_rinv(tc_):
            """Reciprocals of the softmax denominators for t-chunk tc_."""
            rinv_all = work.tile([128, HPC, 8], F32, tag="rinv", name=f"rinv{tc_}")
            sums_ap = bass.AP(
                tensor=ctxn_sb.tensor,
                offset=ctxn_sb[:, 0, 8 * tc_, DEPTH].offset,
                ap=[ctxn_sb.ap[0], [TT * 128, HPC], [128, 8], [1, 1]],
            )
            nc.vector.reciprocal(rinv_all[:], sums_ap)
            half = work.tile([128, 16], F32, tag="half", name=f"half{tc_}")
            return rinv_all, half

        ln = ctx.enter_context(tc.tile_pool(name="ln", bufs=4))

        def phase5_g(tc_, g, rinv_all, half):
            """Residual + LN statistics closures for a group of 4 t-tiles:
            one batched gather-mult, then per-tile fused STT ops whose
            accum_out produces sum(x) / sum(x^2) for free (all vector)."""
            tt0 = 8 * tc_ + 4 * g
            j0 = 4 * g
            ctx_gather = bass.AP(
                tensor=ctxn_sb.tensor,
                offset=ctxn_sb[:, 0, tt0, 0].offset,
                ap=[ctxn_sb.ap[0], [128, 4], [TT * 128, HPC], [1, DEPTH]],
            )
            rinv_b = bass.AP(
                tensor=rinv_all.tensor,
                offset=rinv_all[:, 0, j0].offset,
                ap=[rinv_all.ap[0], [1, 4], [8, HPC], [0, DEPTH]],
            )
            x1 = ln.tile([128, 4, DD], BF16, tag="x1")
            x1_4d = bass.AP(
                tensor=x1.tensor,
                offset=x1[:, 0, 0].offset,
                ap=[x1.ap[0], [DD, 4], [DEPTH, HPC], [1, DEPTH]],
            )
            cls = [lambda: nc.vector.tensor_tensor(
                out=x1_4d, in0=ctx_gather, in1=rinv_b, op=AluOp.mult)]
            for k in range(4):
                tt = tt0 + k
                j = j0 + k

                def sttx(tt=tt, j=j, k=k):
                    nc.vector.scalar_tensor_tensor(
                        out=x_sb[:, tt, :], in0=x1[:, k, :], scalar=0.0,
                        in1=qres_sb[:, tt, :], op0=AluOp.add, op1=AluOp.add,
                        accum_out=half[:, j : j + 1],
                    )

                def sttxx(tt=tt, j=j):
                    xx = ln.tile([128, DD], BF16, tag="xx")
                    nc.vector.scalar_tensor_tensor(
                        out=xx[:], in0=x_sb[:, tt, :], scalar=0.0,
                        in1=x_sb[:, tt, :], op0=AluOp.add, op1=AluOp.mult,
                        accum_out=half[:, 8 + j : 9 + j],
                    )

                cls += [sttx, sttxx]
            return cls

        def phase5_ar(tc_, half):
            nc.sync.dma_start(out=stats_dram[tc_][:, 0:16], in_=half[:])
            nc.gpsimd.collective_compute(
                "AllReduce",
                AluOp.add,
                replica_groups=[[0, 1, 2, 3], [4, 5, 6, 7]],
                ins=[stats_dram[tc_][:, 0:16].opt()],
                outs=[ar_dram[tc_][:, 0:16].opt()],
            )

        def phase7_prep(tc_):
            """Wait for AllReduce tc_, compute mu and rstd on the vector
            engine (rsqrt = Quake bit-trick seed + 2 Newton steps; no
            scalar-engine table switches)."""
            gst = work.tile([128, 16], F32, tag="gst", name=f"gst{tc_}")
            nc.sync.dma_start(out=gst[:], in_=ar_dram[tc_][:, 0:16])
            mu_s = mu[:, 8 * tc_ : 8 * tc_ + 8]
            nc.vector.tensor_scalar(
                out=mu_s, in0=gst[:, 0:8], scalar1=1.0 / D, scalar2=None,
                op0=AluOp.mult,
            )
            var = work.tile([128, 8], F32, tag="var", name=f"var{tc_}")
            nc.vector.tensor_tensor(out=var[:], in0=mu_s, in1=mu_s, op=AluOp.mult)
            # var = ex2 - mu^2 + eps = gst/D - mu^2 + eps
            nc.vector.scalar_tensor_tensor(
                out=var[:], in0=gst[:, 8:16], scalar=1.0 / D, in1=var[:],
                op0=AluOp.mult, op1=AluOp.subtract,
            )
            nc.vector.tensor_scalar(
                out=var[:], in0=var[:], scalar1=LN_EPS, scalar2=None, op0=AluOp.add
            )
            # y0 = bits_f32(magic - bits(v)/2)
            y = work.tile([128, 8], F32, tag="y", name=f"y{tc_}")
            nc.vector.tensor_scalar(
                out=y[:].bitcast(mybir.dt.int32),
                in0=var[:].bitcast(mybir.dt.int32),
                scalar1=-0.5, scalar2=float(0x5F3759DF),
                op0=AluOp.mult, op1=AluOp.add,
            )
            # two Newton steps: y <- y * (1.5 - 0.5 v y^2)
            tn = work.tile([128, 8], F32, tag="tn", name=f"tn{tc_}")
            for it in range(2):
                nc.vector.tensor_tensor(out=tn[:], in0=var[:], in1=y[:], op=AluOp.mult)
                nc.vector.tensor_tensor(out=tn[:], in0=tn[:], in1=y[:], op=AluOp.mult)
                nc.vector.tensor_scalar(
                    out=tn[:], in0=tn[:], scalar1=-0.5, scalar2=1.5,
                    op0=AluOp.mult, op1=AluOp.add,
                )
                dst = rstd[:, 8 * tc_ : 8 * tc_ + 8] if it == 1 else y[:]
                nc.vector.tensor_tensor(out=dst, in0=y[:], in1=tn[:], op=AluOp.mult)

        def phase7_g(tc_, g):
            """Normalize closures for 4 t-tiles: per-tile fused STT pairs
            (x-mu)*gamma then *rstd+beta, all vector engine."""
            tt0 = 8 * tc_ + 4 * g
            cls = []
            for k in range(4):
                tt = tt0 + k

                def p7(tt=tt):
                    s1 = ln.tile([128, DD], BF16, tag="s1")
                    nc.vector.scalar_tensor_tensor(
                        out=s1[:], in0=x_sb[:, tt, :], scalar=mu[:, tt : tt + 1],
                        in1=gam_sb[:], op0=AluOp.subtract, op1=AluOp.mult,
                    )
                    nc.vector.scalar_tensor_tensor(
                        out=out_sb[:, tt, :], in0=s1[:], scalar=rstd[:, tt : tt + 1],
                        in1=bet_sb[:], op0=AluOp.mult, op1=AluOp.add,
                    )

                cls.append(p7)
            return cls

        def phase7_out(tc_):
            nc.gpsimd.dma_start(
                out=out[1024 * tc_ : 1024 * (tc_ + 1), :].rearrange(
                    "(tt p) c -> p tt c", p=128
                ),
                in_=out_sb[:, 8 * tc_ : 8 * tc_ + 8, :],
            )

        # tc0 attention; chunk-0 stats threaded through heads 0-1 of tc1
        # so AR(0) (~10-35us) hides under heads 2-3.  Chunk-1 stats +
        # AR(1) trigger are emitted before any AR(0)-dependent work so
        # the engine FIFOs never block them; p7(0) then overlaps AR(1).
        for hd in range(HPC):
            attn_head(hd, 0, "v" if hd % 2 else "s", act_mod=2)
        r0_, h0_ = phase5_rinv(0)
        f5a = phase5_g(0, 0, r0_, h0_)
        f5b = phase5_g(0, 1, r0_, h0_)
        attn_head(0, 1, "s", act_mod=2, fillers=f5a)
        attn_head(1, 1, "s", act_mod=2, fillers=f5b)
        phase5_ar(0, h0_)
        attn_head(2, 1, "s", act_mod=2)
        attn_head(3, 1, "v", act_mod=2)
        r1_, h1_ = phase5_rinv(1)
        for cl in phase5_g(1, 0, r1_, h1_) + phase5_g(1, 1, r1_, h1_):
            cl()
        phase5_ar(1, h1_)
        phase7_prep(0)
        for cl in phase7_g(0, 0) + phase7_g(0, 1):
            cl()
        phase7_out(0)
        phase7_prep(1)
        for cl in phase7_g(1, 0) + phase7_g(1, 1):
            cl()
        phase7_out(1)

    _split_multi_waits(nc)
    return nc


_CACHE = {}
_LAST_IN_MAPS = None


def kernel(q, k, v, mask, causality, edge_fea, wq, bq, wk, bk, wv, bv, gamma, beta):
    # NB: the reference masks attention row (head eta, batch beta) with
    # mask[eta // 8]; with 4 heads per core this is mask[hg // 2].
    q = np.asarray(q, np.float32)
    k = np.asarray(k, np.float32)
    v = np.asarray(v, np.float32)
    mask = np.asarray(mask)
    wq = np.asarray(wq, np.float32)
    bq = np.asarray(bq, np.float32)
    wk = np.asarray(wk, np.float32)
    bk = np.asarray(bk, np.float32)
    wv = np.asarray(wv, np.float32)
    bv = np.asarray(bv, np.float32)
    gamma = np.asarray(gamma, np.float32)
    beta = np.asarray(beta, np.float32)
    assert int(np.asarray(causality)) == 0

    keep = [np.flatnonzero(mask[g] == 0) for g in range(2)]
    slens = [len(kp) for kp in keep]
    SP = max(128, ((max(slens) + 127) // 128) * 128)
    NS = SP // 128

    qT = [np.ascontiguousarray(q[b].T).astype(BF) for b in range(2)]
    kTc, vTc = {}, {}
    for b in range(2):
        for g in range(2):
            kk = np.zeros((D, SP), BF)
            kk[:, : slens[g]] = k[b][keep[g]].T.astype(BF)
            kTc[b, g] = kk
            vv = np.zeros((DV, SP), BF)
            vv[:D, : slens[g]] = v[b][keep[g]].T.astype(BF)
            vv[D, : slens[g]] = BF(1.0)
            # s-tile-major blocks: [NS, DV, 128] -> [NS*DV, 128]
            vTc[b, g] = np.ascontiguousarray(
                vv.reshape(DV, NS, 128).transpose(1, 0, 2).reshape(NS * DV, 128)
            )

    in_maps = []
    for c in range(8):
        b, hg = c // 4, c % 4
        g = hg // 2
        c0 = hg * DD
        wvp = np.zeros((DV, HPC * (DEPTH + 1)), BF)
        for hh in range(HPC):
            wvp[:D, hh * (DEPTH + 1) : hh * (DEPTH + 1) + DEPTH] = (
                wv[c0 + hh * DEPTH : c0 + (hh + 1) * DEPTH].T.astype(BF)
            )
            wvp[D, hh * (DEPTH + 1) + DEPTH] = BF(1.0)
        in_maps.append(
            {
                "qT": qT[b],
                "kT": kTc[b, g],
                "vTs": vTc[b, g],
                "wqT": np.ascontiguousarray(wq[c0 : c0 + DD].T).astype(BF),
                "wkT": np.ascontiguousarray(wk[c0 : c0 + DD].T).astype(BF),
                "wvT": wvp,
                "bq": np.ascontiguousarray(bq[c0 : c0 + DD].reshape(2, 128).T),
                "bk": np.ascontiguousarray(bk[c0 : c0 + DD].reshape(2, 128).T),
                "qres": (q[b][:, c0 : c0 + DD] + bv[c0 : c0 + DD]).astype(BF),
                "gamma": gamma[c0 : c0 + DD].reshape(1, DD).astype(BF),
                "beta": beta[c0 : c0 + DD].reshape(1, DD).astype(BF),
            }
        )

    global _LAST_IN_MAPS
    _LAST_IN_MAPS = in_maps
    if SP not in _CACHE:
        _CACHE[SP] = _build(SP)
    nc = _CACHE[SP]

    res = run_bass_kernel_spmd(nc, in_maps, list(range(8))).results

    full = np.empty((B, T, D), np.float32)
    for c in range(8):
        b, hg = c // 4, c % 4
        full[b, :, hg * DD : (hg + 1) * DD] = np.asarray(res[c]["out"], np.float32)
    return full


# revision 28
# speedup vs baseline: 1.0808x; 1.0808x over previous
"""Trainium2 Bass kernel for nn_MultiHeadAttention_7516192768586.

Full MHA: QKV projection -> masked softmax attention -> merge heads ->
residual add -> LayerNorm.  B=2, T=2048, D=1024, 16 heads (depth 64).

Sharding (8 cores): 2 batches x 4 head-groups (4 heads each, i.e. a
256-channel slice of the projected dims).  Each core computes attention for
its 4 heads, the residual+LayerNorm for its 256 output *columns* over all
2048 rows of its batch; LayerNorm statistics (sum x, sum x^2 over the full
1024 channels) are combined with a tiny 8KB-per-chunk AllReduce within each
batch-group of 4 cores.  Host reassembles the 8 (2048, 256) column slices.

Key points:
  * q/k/v fed transposed (contraction dim D on partitions) as bf16; v is
    fed s-tile-major so compute can start as soon as the first block lands
  * masked keys are compacted away on the host; compaction padding columns
    are all-zero so they contribute exp(0)*0 = 0 to both the context
    numerator and the denominator (no pad bias needed at all)
  * the reference's jnp.repeat(mask, h, axis=0) is batch-major while its
    head stacking is head-major, so attention row (head eta, batch beta)
    is masked by mask[eta // 8] -- replicated here faithfully
  * V's weight matrix gets one extra "ones" output channel per head so the
    attention-context matmul also produces the softmax denominators
  * V's bias is folded into the residual (ctx = sum attn (V+bv) = ctx0 +
    bv since attn rows sum to 1)
  * exp is split between the Scalar engine (exact, table-based) and the
    Vector engine (Schraudolph bit-trick: int16(x*s+b) bitcast to bf16,
    ~1.6% mean error -- harmless after the softmax normalization since the
    output tolerance is 2e-2 and ctx ~ 0.05 x output scale)
  * score matmuls for the two heads of a K-partition (rows 0-63 / 64-127)
    are issued back-to-back so the PE runs them concurrently in distinct
    row-groups
  * LayerNorm is pipelined per 1024-row chunk: fused scalar_tensor_tensor
    ops (with accum_out producing sum-x / sum-x^2 for free) run on the
    otherwise idle GpSimd engine, the per-chunk AllReduce overlaps the
    next chunk's attention, and 1/sqrt(var+eps) is exp(-0.5*ln(var+eps))
    so only one activation-table set is ever loaded
"""

import sys

if "/opt/trn_rl_repo" not in sys.path:
    sys.path.insert(0, "/opt/trn_rl_repo")

import contextlib

import ml_dtypes
import numpy as np

import bass_rust as _br
import concourse.bass as bass
import concourse.tile as tile
from concourse import mybir
from concourse.bass_utils import run_bass_kernel_spmd
from concourse.vector_clock import ScopedClock

F32 = mybir.dt.float32
BF16 = mybir.dt.bfloat16
I16 = mybir.dt.int16
BF = ml_dtypes.bfloat16

NUM_HEADS = 16
LN_EPS = 1e-5
B, T, D = 2, 2048, 1024
DEPTH = D // NUM_HEADS  # 64
HPC = 4  # heads per core
DD = HPC * DEPTH  # 256 projected channels per core
DV = 1152  # v contraction dim padded: 1024 + ones row + zeros (9 k-tiles)
NKV = DV // 128  # 9
TT = T // 128  # 16 t-tiles
AluOp = mybir.AluOpType
Act = mybir.ActivationFunctionType

# Schraudolph exp on bf16 bit patterns: e^(x/8) ~= bf16_bits(int16(x*ES + EB))
ES = 0.125 * 128.0 * float(np.log2(np.e))
EB = 127.0 * 128.0 - 7.0


class _TC(tile.TileContext):
    """TileContext whose tail drain splits its sem waits across 1-wait NOPs
    (this walrus build rejects >1 sync wait on one instruction)."""

    def _drain_and_barrier(self, tick_clock, wait_clock):
        nc = self.nc
        drain_inst = nc.sync.drain()
        wait_clock.add_sem_waits(
            drain_inst.ins, ScopedClock({None: tick_clock.global_clock})
        )
        si = drain_inst.ins.sync_info
        waits = list(si.on_wait) if si is not None and si.on_wait else []
        if len(waits) > 1:
            si.on_wait = waits[:1]
            for i in range(1, len(waits)):
                extra = nc.sync.nop()
                extra.ins.sync_info = _br.SyncInfo(
                    on_wait=waits[i : i + 1], on_update=[]
                )
        nc.all_engine_barrier()
        popped = nc._tile_sem_poison_stack.pop()
        assert popped is self._sem_poison
        assert self.sems is not None
        nc.clear_and_free_semaphores(list(self.sems.allocated().values()))
        nc.all_engine_barrier()


def _split_multi_waits(nc):
    """Move extra sem waits (>1 per instruction) onto same-engine NOPs
    inserted immediately before the instruction."""
    f = nc.m.functions[0]
    cur_bb = nc.cur_bb
    for block in f.blocks:
        insts = list(block.instructions)
        if not any(
            i.sync_info is not None
            and i.sync_info.on_wait
            and len(i.sync_info.on_wait) > 1
            for i in insts
        ):
            continue
        new_list = []
        for inst in insts:
            si = inst.sync_info
            if si is not None and si.on_wait and len(si.on_wait) > 1:
                waits = list(si.on_wait)
                si.on_wait = waits[:1]
                for w in waits[1:]:
                    eng = nc.engines[inst.engine]
                    nop = eng.nop()
                    tail_bb = cur_bb.bb if hasattr(cur_bb, "bb") else cur_bb
                    tl = list(tail_bb.instructions)
                    assert tl and tl[-1].name == nop.ins.name
                    tail_bb.instructions = tl[:-1]
                    nop.ins.sync_info = _br.SyncInfo(on_wait=[w], on_update=[])
                    new_list.append(nop.ins)
            new_list.append(inst)
        block.instructions = new_list


def _build(SP):
    """Build the per-core Bass program. SP = padded compacted key count."""
    NS = SP // 128  # s-tiles
    NKCH = (SP + 511) // 512  # 512-wide chunks of SP for the K projection

    nc = bass.Bass("TRN2", target_bir_lowering=False, debug=False, num_devices=8)

    p = lambda name, shape, dt: nc.declare_dram_parameter(name, shape, dt, isOutput=False)
    qT = p("qT", [D, T], BF16)
    kT = p("kT", [D, SP], BF16)
    vTs = p("vTs", [NS * DV, 128], BF16)  # s-tile-major blocks of v^T
    wqT = p("wqT", [D, DD], BF16)
    wkT = p("wkT", [D, DD], BF16)
    wvT = p("wvT", [DV, HPC * (DEPTH + 1)], BF16)
    bq = p("bq", [128, 2], F32)
    bk = p("bk", [128, 2], F32)
    qres = p("qres", [T, DD], BF16)
    gam = p("gamma", [1, DD], BF16)
    bet = p("beta", [1, DD], BF16)
    out = nc.declare_dram_parameter("out", [T, DD], BF16, isOutput=True)

    with _TC(nc) as tc, contextlib.ExitStack() as ctx:
        singles = ctx.enter_context(tc.tile_pool(name="singles", bufs=1))
        persist = ctx.enter_context(tc.tile_pool(name="persist", bufs=1))
        work = ctx.enter_context(tc.tile_pool(name="work", bufs=4))
        dram = ctx.enter_context(tc.tile_pool(name="dram", bufs=1, space="DRAM"))

        # ---- constants / weights (in consumption order) ----
        wv_sb = singles.tile([128, NKV, HPC * (DEPTH + 1)], BF16)
        nc.sync.dma_start(out=wv_sb[:], in_=wvT[:].rearrange("(kt p) c -> p kt c", p=128))
        bq_sb = singles.tile([128, 2], F32)
        nc.gpsimd.dma_start(out=bq_sb[:], in_=bq[:])
        bk_sb = singles.tile([128, 2], F32)
        nc.gpsimd.dma_start(out=bk_sb[:], in_=bk[:])
        gam_sb = singles.tile([128, DD], BF16)
        g_ap = gam[:]
        nc.gpsimd.dma_start(
            out=gam_sb[:],
            in_=bass.AP(tensor=g_ap.tensor, offset=g_ap.offset, ap=[[0, 128], list(g_ap.ap[-1])]),
        )
        bet_sb = singles.tile([128, DD], BF16)
        b_ap = bet[:]
        nc.gpsimd.dma_start(
            out=bet_sb[:],
            in_=bass.AP(tensor=b_ap.tensor, offset=b_ap.offset, ap=[[0, 128], list(b_ap.ap[-1])]),
        )
        eps_sb = singles.tile([128, 1], F32)
        nc.vector.memset(eps_sb[:], LN_EPS)
        # warm the scalar-engine exp table during the initial DMA wait
        warm = singles.tile([128, 1], F32)
        nc.vector.memset(warm[:], 1.0)
        nc.scalar.activation(out=warm[:], in_=warm[:], func=Act.Exp)
        # warm the collective path: the first AllReduce of a NEFF pays a
        # ~11.5us setup delay; also acts as a soft cross-core sync
        warm_d = dram.tile([128, 1], F32, name="warmd")
        warm_a = dram.tile([128, 1], F32, name="warma")
        nc.sync.dma_start(out=warm_d[:], in_=warm[:])
        nc.gpsimd.collective_compute(
            "AllReduce",
            AluOp.add,
            replica_groups=[[0, 1, 2, 3], [4, 5, 6, 7]],
            ins=[warm_d[:].opt()],
            outs=[warm_a[:].opt()],
        )

        # ---- persistent activations ----
        QT_sb = persist.tile([128, 2, T], BF16)  # [dd-in-tile, ddt, t]
        KT_sb = persist.tile([128, 2, SP], BF16)
        VH_sb = persist.tile([128, NS, HPC * (DEPTH + 1)], BF16)  # [s, st, head*65+c]
        ctxT_sb = persist.tile([128, HPC, T], BF16)  # rows 0..64 valid
        nc.gpsimd.memset(ctxT_sb[:], 0.0)
        ctxn_sb = persist.tile([128, HPC, TT, 128], BF16)  # transposed ctx
        x_sb = persist.tile([128, TT, DD], BF16)  # residual+ctx, pre-norm
        out_sb = persist.tile([128, TT, DD], BF16)  # normalized output
        mu = singles.tile([128, TT], F32)
        rstd = singles.tile([128, TT], F32)

        # ---- input streaming: all big input DMAs up front (sync/HWDGE),
        # in consumption order: vTs, wk, kT, wq, qT, qres ----
        ain = ctx.enter_context(tc.tile_pool(name="ain", bufs=8))
        vin = ctx.enter_context(tc.tile_pool(name="vin", bufs=NS))
        vts = []
        for st in range(NS):
            t_ = vin.tile([128, NKV, 128], BF16, tag="vin", name="vt")
            nc.sync.dma_start(
                out=t_[:],
                in_=vTs[DV * st : DV * (st + 1), :].rearrange(
                    "(kt p) s -> p kt s", p=128
                ),
            )
            vts.append(t_)
        wk_sb = singles.tile([128, 8, DD], BF16)
        nc.sync.dma_start(out=wk_sb[:], in_=wkT[:].rearrange("(kt p) c -> p kt c", p=128))
        kin = []
        for kt in range(8):
            t_ = ain.tile([128, SP], BF16, tag="kin", name="kin")
            nc.sync.dma_start(out=t_[:], in_=kT[128 * kt : 128 * (kt + 1), :])
            kin.append(t_)
        wq_sb = singles.tile([128, 8, DD], BF16)
        nc.sync.dma_start(out=wq_sb[:], in_=wqT[:].rearrange("(kt p) c -> p kt c", p=128))
        qin = []
        for kt in range(8):
            t_ = ain.tile([128, T], BF16, tag="qin", name="qin")
            nc.sync.dma_start(out=t_[:], in_=qT[128 * kt : 128 * (kt + 1), :])
            qin.append(t_)
        qres_sb = persist.tile([128, TT, DD], BF16)
        nc.sync.dma_start(
            out=qres_sb[:], in_=qres[:].rearrange("(tt p) c -> p tt c", p=128)
        )

        # ---- V projection (own PSUM scope, closed before attention) ----
        with tc.tile_pool(name="pv", bufs=2, space="PSUM") as pv:
            for st in range(NS):
                ps = pv.tile([128, HPC * (DEPTH + 1)], F32, tag="pv")
                for kt in range(NKV):
                    nc.tensor.matmul(
                        ps[:],
                        vts[st][:, kt, :],
                        wv_sb[:, kt, :],
                        start=(kt == 0),
                        stop=(kt == NKV - 1),
                    )
                if st % 2 == 0:
                    nc.vector.tensor_copy(VH_sb[:, st, :], ps[:])
                else:
                    nc.scalar.copy(VH_sb[:, st, :], ps[:])

        # ---- K/Q projections (own PSUM scope) ----
        with tc.tile_pool(name="pqk", bufs=3, space="PSUM") as pqk:

            def k_proj(ddt):
                for sch in range(NKCH):
                    w = min(512, SP - 512 * sch)
                    ps = pqk.tile([128, 512], F32, tag="pqk", name="kps")
                    for kt in range(8):
                        nc.tensor.matmul(
                            ps[:, :w],
                            wk_sb[:, kt, 128 * ddt : 128 * (ddt + 1)],
                            kin[kt][:, 512 * sch : 512 * sch + w],
                            start=(kt == 0),
                            stop=(kt == 7),
                        )
                    dst = KT_sb[:, ddt, 512 * sch : 512 * sch + w]
                    if sch % 2 == 0:
                        nc.vector.tensor_scalar(
                            out=dst, in0=ps[:, :w],
                            scalar1=bk_sb[:, ddt : ddt + 1], scalar2=None,
                            op0=AluOp.add,
                        )
                    else:
                        nc.scalar.activation(
                            out=dst, in_=ps[:, :w], func=Act.Identity,
                            bias=bk_sb[:, ddt : ddt + 1], scale=1.0,
                        )

            def q_proj(ddt):
                for tch in range(4):
                    ps = pqk.tile([128, 512], F32, tag="pqk", name="qps")
                    for kt in range(8):
                        nc.tensor.matmul(
                            ps[:],
                            wq_sb[:, kt, 128 * ddt : 128 * (ddt + 1)],
                            qin[kt][:, 512 * tch : 512 * (tch + 1)],
                            start=(kt == 0),
                            stop=(kt == 7),
                        )
                    dst = QT_sb[:, ddt, 512 * tch : 512 * (tch + 1)]
                    if tch % 2 == 0:
                        nc.scalar.activation(
                            out=dst, in_=ps[:], func=Act.Identity,
                            bias=bq_sb[:, ddt : ddt + 1], scale=1.0,
                        )
                    else:
                        nc.vector.tensor_scalar(
                            out=dst, in0=ps[:],
                            scalar1=bq_sb[:, ddt : ddt + 1], scalar2=None,
                            op0=AluOp.add,
                        )

            k_proj(0)
            q_proj(0)
            k_proj(1)
            q_proj(1)

        # ---- attention + pipelined LN ----
        stats_dram = [dram.tile([128, TT], F32, name=f"std{i}") for i in range(2)]
        ar_dram = [dram.tile([128, TT], F32, name=f"ard{i}") for i in range(2)]
        epool = ctx.enter_context(tc.tile_pool(name="epool", bufs=6))
        # PSUM: 3 score tiles (2 banks each, depth-3 pipeline) + 1 ctx (2) = 8
        pscore = ctx.enter_context(tc.tile_pool(name="pscore", bufs=3, space="PSUM"))
        pctx = ctx.enter_context(tc.tile_pool(name="pctx", bufs=1, space="PSUM"))

        # exp engine schedule: every act_mod-th tile on the scalar engine
        # (exact), the rest on the vector engine (bit-trick)
        ecnt = [0]

        def emit_exp(e_ap, s_ap, act_mod):
            i = ecnt[0]
            ecnt[0] += 1
            if i % act_mod == act_mod - 1:
                nc.scalar.activation(out=e_ap, in_=s_ap, func=Act.Exp, scale=0.125)
            else:
                nc.vector.tensor_scalar(
                    out=e_ap.bitcast(I16), in0=s_ap,
                    scalar1=ES, scalar2=EB,
                    op0=AluOp.mult, op1=AluOp.add,
                )

        def attn_head(hd, tc_, copy_eng, act_mod=3, fillers=()):
            """Attention for head hd over t-chunk tc_ (1024 queries).

            Software-pipelined: scores(st) and exp(st) are emitted before
            ctx(st-1) so the PE streams scores while ACT/DVE exponentiate
            and the 3-deep score-psum pool absorbs exp latency jitter.
            ``fillers``: closures emitted one-per-st (spread LN work from
            earlier chunks through this head's vector-engine stream)."""
            t0 = 1024 * tc_
            ddt, h = hd // 2, hd % 2
            r0 = DEPTH * h
            cps = pctx.tile([DEPTH + 1, 1024], F32, tag="cps", name="cps")
            es = [None] * NS
            fillers = list(fillers)

            def emit_ctx(st):
                for q2 in range(2):
                    nc.tensor.matmul(
                        cps[:, 512 * q2 : 512 * (q2 + 1)],
                        VH_sb[:, st, (DEPTH + 1) * hd : (DEPTH + 1) * (hd + 1)],
                        es[st][:, 512 * q2 : 512 * (q2 + 1)],
                        start=(st == 0),
                        stop=(st == NS - 1),
                    )

            for st in range(NS):
                s_ = pscore.tile([128, 1024], F32, tag="sc", name="s")
                for q2 in range(2):
                    nc.tensor.matmul(
                        s_[:, 512 * q2 : 512 * (q2 + 1)],
                        KT_sb[r0 : r0 + DEPTH, ddt, 128 * st : 128 * (st + 1)],
                        QT_sb[r0 : r0 + DEPTH, ddt, t0 + 512 * q2 : t0 + 512 * (q2 + 1)],
                        start=True,
                        stop=True,
                    )
                e_ = epool.tile([128, 1024], BF16, tag="e")
                emit_exp(e_[:], s_[:], act_mod)
                es[st] = e_
                if st > 0:
                    emit_ctx(st - 1)
                if fillers:
                    fillers.pop(0)()
            emit_ctx(NS - 1)
            for f in fillers:
                f()

            dst = ctxT_sb[0 : DEPTH + 1, hd, t0 : t0 + 1024]
            if copy_eng == "v":
                nc.vector.tensor_copy(dst, cps[:])
            else:
                nc.scalar.copy(dst, cps[:])
            nc.sync.dma_start_transpose(
                ctxn_sb[:, hd, 8 * tc_ : 8 * (tc_ + 1), :],
                ctxT_sb[:, hd, t0 : t0 + 1024],
            )

        def phase5_rinv(tc_):
            """Reciprocals of the softmax denominators for t-chunk tc_."""
            rinv_all = work.tile([128, HPC, 8], F32, tag="rinv", name=f"rinv{tc_}")
            sums_ap = bass.AP(
                tensor=ctxn_sb.tensor,
                offset=ctxn_sb[:, 0, 8 * tc_, DEPTH].offset,
                ap=[ctxn_sb.ap[0], [TT * 128, HPC], [128, 8], [1, 1]],
            )
            nc.vector.reciprocal(rinv_all[:], sums_ap)
            half = work.tile([128, 16], F32, tag="half", name=f"half{tc_}")
            return rinv_all, half

        ln = ctx.enter_context(tc.tile_pool(name="ln", bufs=4))

        def phase5_g(tc_, g, rinv_all, half):
            """Residual + LN statistics closures for a group of 4 t-tiles:
            one batched gather-mult, then per-tile fused STT ops whose
            accum_out produces sum(x) / sum(x^2) for free (all vector)."""
            tt0 = 8 * tc_ + 4 * g
            j0 = 4 * g
            ctx_gather = bass.AP(
                tensor=ctxn_sb.tensor,
                offset=ctxn_sb[:, 0, tt0, 0].offset,
                ap=[ctxn_sb.ap[0], [128, 4], [TT * 128, HPC], [1, DEPTH]],
            )
            rinv_b = bass.AP(
                tensor=rinv_all.tensor,
                offset=rinv_all[:, 0, j0].offset,
                ap=[rinv_all.ap[0], [1, 4], [8, HPC], [0, DEPTH]],
            )
            x1 = ln.tile([128, 4, DD], BF16, tag="x1")
            x1_4d = bass.AP(
                tensor=x1.tensor,
                offset=x1[:, 0, 0].offset,
                ap=[x1.ap[0], [DD, 4], [DEPTH, HPC], [1, DEPTH]],
            )
            cls = [lambda: nc.vector.tensor_tensor(
                out=x1_4d, in0=ctx_gather, in1=rinv_b, op=AluOp.mult)]
            for k in range(4):
                tt = tt0 + k
                j = j0 + k

                def sttx(tt=tt, j=j, k=k):
                    nc.vector.scalar_tensor_tensor(
                        out=x_sb[:, tt, :], in0=x1[:, k, :], scalar=0.0,
                        in1=qres_sb[:, tt, :], op0=AluOp.add, op1=AluOp.add,
                        accum_out=half[:, j : j + 1],
                    )

                def sttxx(tt=tt, j=j):
                    xx = ln.tile([128, DD], BF16, tag="xx")
                    nc.vector.scalar_tensor_tensor(
                        out=xx[:], in0=x_sb[:, tt, :], scalar=0.0,
                        in1=x_sb[:, tt, :], op0=AluOp.add, op1=AluOp.mult,
                        accum_out=half[:, 8 + j : 9 + j],
                    )

                cls += [sttx, sttxx]
            return cls

        def phase5_ar(tc_, half):
            nc.sync.dma_start(out=stats_dram[tc_][:, 0:16], in_=half[:])
            nc.gpsimd.collective_compute(
                "AllReduce",
                AluOp.add,
                replica_groups=[[0, 1, 2, 3], [4, 5, 6, 7]],
                ins=[stats_dram[tc_][:, 0:16].opt()],
                outs=[ar_dram[tc_][:, 0:16].opt()],
            )

        def phase7_prep(tc_):
            """Wait for AllReduce tc_, compute mu and rstd on the vector
            engine (rsqrt = Quake bit-trick seed + 2 Newton steps; no
            scalar-engine table switches)."""
            gst = work.tile([128, 16], F32, tag="gst", name=f"gst{tc_}")
            nc.sync.dma_start(out=gst[:], in_=ar_dram[tc_][:, 0:16])
            mu_s = mu[:, 8 * tc_ : 8 * tc_ + 8]
            nc.vector.tensor_scalar(
                out=mu_s, in0=gst[:, 0:8], scalar1=1.0 / D, scalar2=None,
                op0=AluOp.mult,
            )
            var = work.tile([128, 8], F32, tag="var", name=f"var{tc_}")
            nc.vector.tensor_tensor(out=var[:], in0=mu_s, in1=mu_s, op=AluOp.mult)
            # var = ex2 - mu^2 + eps = gst/D - mu^2 + eps
            nc.vector.scalar_tensor_tensor(
                out=var[:], in0=gst[:, 8:16], scalar=1.0 / D, in1=var[:],
                op0=AluOp.mult, op1=AluOp.subtract,
            )
            nc.vector.tensor_scalar(
                out=var[:], in0=var[:], scalar1=LN_EPS, scalar2=None, op0=AluOp.add
            )
            # y0 = bits_f32(magic - bits(v)/2)
            y = work.tile([128, 8], F32, tag="y", name=f"y{tc_}")
            nc.vector.tensor_scalar(
                out=y[:].bitcast(mybir.dt.int32),
                in0=var[:].bitcast(mybir.dt.int32),
                scalar1=-0.5, scalar2=float(0x5F3759DF),
                op0=AluOp.mult, op1=AluOp.add,
            )
            # two Newton steps: y <- y * (1.5 - 0.5 v y^2)
            tn = work.tile([128, 8], F32, tag="tn", name=f"tn{tc_}")
            for it in range(2):
                nc.vector.tensor_tensor(out=tn[:], in0=var[:], in1=y[:], op=AluOp.mult)
                nc.vector.tensor_tensor(out=tn[:], in0=tn[:], in1=y[:], op=AluOp.mult)
                nc.vector.tensor_scalar(
                    out=tn[:], in0=tn[:], scalar1=-0.5, scalar2=1.5,
                    op0=AluOp.mult, op1=AluOp.add,
                )
                dst = rstd[:, 8 * tc_ : 8 * tc_ + 8] if it == 1 else y[:]
                nc.vector.tensor_tensor(out=dst, in0=y[:], in1=tn[:], op=AluOp.mult)

        def phase7_g(tc_, g):
            """Normalize closures for 4 t-tiles: per-tile fused STT pairs
            (x-mu)*gamma then *rstd+beta, all vector engine."""
            tt0 = 8 * tc_ + 4 * g
            cls = []
            for k in range(4):
                tt = tt0 + k

                def p7(tt=tt):
                    s1 = ln.tile([128, DD], BF16, tag="s1")
                    nc.vector.scalar_tensor_tensor(
                        out=s1[:], in0=x_sb[:, tt, :], scalar=mu[:, tt : tt + 1],
                        in1=gam_sb[:], op0=AluOp.subtract, op1=AluOp.mult,
                    )
                    nc.vector.scalar_tensor_tensor(
                        out=out_sb[:, tt, :], in0=s1[:], scalar=rstd[:, tt : tt + 1],
                        in1=bet_sb[:], op0=AluOp.mult, op1=AluOp.add,
                    )

                cls.append(p7)
            return cls

        def phase7_out(tc_):
            nc.gpsimd.dma_start(
                out=out[1024 * tc_ : 1024 * (tc_ + 1), :].rearrange(
                    "(tt p) c -> p tt c", p=128
                ),
                in_=out_sb[:, 8 * tc_ : 8 * tc_ + 8, :],
            )

        # tc0 attention; chunk-0 stats threaded through heads 0-1 of tc1
        # so AR(0) (~10-35us) hides under heads 2-3.  Chunk-1 stats +
        # AR(1) trigger are emitted before any AR(0)-dependent work so
        # the engine FIFOs never block them; p7(0) then overlaps AR(1).
        for hd in range(HPC):
            attn_head(hd, 0, "v" if hd % 2 else "s", act_mod=2)
        r0_, h0_ = phase5_rinv(0)
        f5a = phase5_g(0, 0, r0_, h0_)
        f5b = phase5_g(0, 1, r0_, h0_)
        attn_head(0, 1, "s", act_mod=2, fillers=f5a)
        attn_head(1, 1, "s", act_mod=2, fillers=f5b)
        phase5_ar(0, h0_)
        attn_head(2, 1, "s", act_mod=2)
        attn_head(3, 1, "s", act_mod=2)
        r1_, h1_ = phase5_rinv(1)
        for cl in phase5_g(1, 0, r1_, h1_) + phase5_g(1, 1, r1_, h1_):
            cl()
        phase5_ar(1, h1_)
        phase7_prep(0)
        for cl in phase7_g(0, 0) + phase7_g(0, 1):
            cl()
        phase7_out(0)
        phase7_prep(1)
        for cl in phase7_g(1, 0) + phase7_g(1, 1):
            cl()
        phase7_out(1)

    _split_multi_waits(nc)
    return nc


_CACHE = {}
_LAST_IN_MAPS = None


def kernel(q, k, v, mask, causality, edge_fea, wq, bq, wk, bk, wv, bv, gamma, beta):
    # NB: the reference masks attention row (head eta, batch beta) with
    # mask[eta // 8]; with 4 heads per core this is mask[hg // 2].
    q = np.asarray(q, np.float32)
    k = np.asarray(k, np.float32)
    v = np.asarray(v, np.float32)
    mask = np.asarray(mask)
    wq = np.asarray(wq, np.float32)
    bq = np.asarray(bq, np.float32)
    wk = np.asarray(wk, np.float32)
    bk = np.asarray(bk, np.float32)
    wv = np.asarray(wv, np.float32)
    bv = np.asarray(bv, np.float32)
    gamma = np.asarray(gamma, np.float32)
    beta = np.asarray(beta, np.float32)
    assert int(np.asarray(causality)) == 0

    keep = [np.flatnonzero(mask[g] == 0) for g in range(2)]
    slens = [len(kp) for kp in keep]
    SP = max(128, ((max(slens) + 127) // 128) * 128)
    NS = SP // 128

    qT = [np.ascontiguousarray(q[b].T).astype(BF) for b in range(2)]
    kTc, vTc = {}, {}
    for b in range(2):
        for g in range(2):
            kk = np.zeros((D, SP), BF)
            kk[:, : slens[g]] = k[b][keep[g]].T.astype(BF)
            kTc[b, g] = kk
            vv = np.zeros((DV, SP), BF)
            vv[:D, : slens[g]] = v[b][keep[g]].T.astype(BF)
            vv[D, : slens[g]] = BF(1.0)
            # s-tile-major blocks: [NS, DV, 128] -> [NS*DV, 128]
            vTc[b, g] = np.ascontiguousarray(
                vv.reshape(DV, NS, 128).transpose(1, 0, 2).reshape(NS * DV, 128)
            )

    in_maps = []
    for c in range(8):
        b, hg = c // 4, c % 4
        g = hg // 2
        c0 = hg * DD
        wvp = np.zeros((DV, HPC * (DEPTH + 1)), BF)
        for hh in range(HPC):
            wvp[:D, hh * (DEPTH + 1) : hh * (DEPTH + 1) + DEPTH] = (
                wv[c0 + hh * DEPTH : c0 + (hh + 1) * DEPTH].T.astype(BF)
            )
            wvp[D, hh * (DEPTH + 1) + DEPTH] = BF(1.0)
        in_maps.append(
            {
                "qT": qT[b],
                "kT": kTc[b, g],
                "vTs": vTc[b, g],
                "wqT": np.ascontiguousarray(wq[c0 : c0 + DD].T).astype(BF),
                "wkT": np.ascontiguousarray(wk[c0 : c0 + DD].T).astype(BF),
                "wvT": wvp,
                "bq": np.ascontiguousarray(bq[c0 : c0 + DD].reshape(2, 128).T),
                "bk": np.ascontiguousarray(bk[c0 : c0 + DD].reshape(2, 128).T),
                "qres": (q[b][:, c0 : c0 + DD] + bv[c0 : c0 + DD]).astype(BF),
                "gamma": gamma[c0 : c0 + DD].reshape(1, DD).astype(BF),
                "beta": beta[c0 : c0 + DD].reshape(1, DD).astype(BF),
            }
        )

    global _LAST_IN_MAPS
    _LAST_IN_MAPS = in_maps
    if SP not in _CACHE:
        _CACHE[SP] = _build(SP)
    nc = _CACHE[SP]

    res = run_bass_kernel_spmd(nc, in_maps, list(range(8))).results

    full = np.empty((B, T, D), np.float32)
    for c in range(8):
        b, hg = c // 4, c % 4
        full[b, :, hg * DD : (hg + 1) * DD] = np.asarray(res[c]["out"], np.float32)
    return full
